# revision 6
# baseline (speedup 1.0000x reference)
"""GraphConv (DGL norm='both') + log_softmax on 8 Trainium2 NeuronCores.

Strategy (per sharding hint): partition nodes across the 8 cores by range.
  Launch A (per core): project its 12500-node slice m = (h @ W) * out_deg^-1/2.
  Host: concatenate the 8 projected shards into a replicated gather table.
  Launch B (per core): for its 12500 dst nodes, gather m[src] rows for all
  in-edges (dma_gather, edges pre-sorted by dst group), segment-sum via
  one-hot matmuls accumulating in PSUM, then norm/bias/log_softmax.

Degrees and the sorted/padded edge metadata are sharding-prep computed on the
host (numpy); all FLOPs on h/W/b/m (projection, normalization, aggregation,
softmax) run on device.
"""

import numpy as np
import ml_dtypes

import concourse.bass as bass
import concourse.bacc as bacc
import concourse.mybir as mybir
import concourse.tile as tile
from concourse.bass_utils import run_bass_kernel_spmd

P = 128
N_NODES = 100000
N_EDGES = 3200000
IN_DIM = 256
OUT_DIM = 64
NCORES = 8
G = N_NODES // NCORES            # 12500 nodes per core
NG = (G + P - 1) // P            # 98 groups of 128 dst nodes (last has 84)
GPAD = NG * P                    # 12544
NT = 4                           # gather sub-tables (int16 index limit)
TROWS = (NCORES * GPAD) // NT    # 25088 rows per sub-table
ROUND_G = 4                      # dst groups processed per gather round
PAD_LDST = 200.0                 # local-dst for padded edges (>127, exact in bf16)

_f32 = mybir.dt.float32
_bf16 = mybir.dt.bfloat16
_i16 = mybir.dt.int16


# ---------------------------------------------------------------- launch A
def build_launch_a():
    nc = bacc.Bacc("TRN2", target_bir_lowering=False, debug=False,
                   num_devices=NCORES)
    hT = nc.dram_tensor("hT", [NG, IN_DIM, P], _f32, kind="ExternalInput")
    W = nc.dram_tensor("W", [IN_DIM, OUT_DIM], _f32, kind="ExternalInput")
    odeg = nc.dram_tensor("odeg", [P, NG], _f32, kind="ExternalInput")
    m = nc.dram_tensor("m", [GPAD, OUT_DIM], _f32, kind="ExternalOutput")

    with tile.TileContext(nc) as tc:
        with (
            tc.tile_pool(name="const", bufs=1) as cpool,
            tc.tile_pool(name="work", bufs=3) as pool,
            tc.tile_pool(name="psum", bufs=4, space="PSUM") as psum,
        ):
            w0 = cpool.tile([P, OUT_DIM], _f32, tag="w0")
            w1 = cpool.tile([P, OUT_DIM], _f32, tag="w1")
            nc.sync.dma_start(out=w0[:], in_=W[0:P, :])
            nc.sync.dma_start(out=w1[:], in_=W[P:2 * P, :])

            dt_ = cpool.tile([P, NG], _f32, tag="deg")
            norm = cpool.tile([P, NG], _f32, tag="norm")
            nc.sync.dma_start(out=dt_[:], in_=odeg[:, :])
            nc.vector.tensor_scalar_max(out=dt_[:], in0=dt_[:], scalar1=1.0)
            nc.vector.reciprocal(out=dt_[:], in_=dt_[:])
            nc.scalar.sqrt(out=norm[:], in_=dt_[:])

            for g in range(NG):
                l0 = pool.tile([P, P], _f32, tag="l0")
                l1 = pool.tile([P, P], _f32, tag="l1")
                nc.sync.dma_start(out=l0[:], in_=hT[g, 0:P, :])
                nc.sync.dma_start(out=l1[:], in_=hT[g, P:2 * P, :])
                acc = psum.tile([P, OUT_DIM], _f32, tag="acc")
                nc.tensor.matmul(acc[:], l0[:], w0[:], start=True, stop=False)
                nc.tensor.matmul(acc[:], l1[:], w1[:], start=False, stop=True)
                ms = pool.tile([P, OUT_DIM], _f32, tag="ms")
                nc.scalar.activation(out=ms[:], in_=acc[:],
                                     func=mybir.ActivationFunctionType.Identity,
                                     scale=norm[:, g:g + 1])
                nc.sync.dma_start(out=m[g * P:(g + 1) * P, :], in_=ms[:])
    nc.compile()
    return nc


# ---------------------------------------------------------------- launch B
def build_launch_b(meta):
    """meta: dict with the uniform (cross-core) layout:
      rounds: list of rounds; each round is a dict:
        groups: list of group ids g
        q_numidx: [NT] number of indices (incl. padding) per sub-table gather
        q_choff:  [NT] chunk-column offset of each q region in the round tile
        nch:      total chunks in the round tile
        idx_off:  column offset into the gidx int16 DRAM tensor
        ch_off:   column offset into the ldst DRAM tensor
        gcols:    {g: list of (column-in-round-tile) for that group's chunks}
      tot_idx_cols, tot_chunks
    """
    nc = bacc.Bacc("TRN2", target_bir_lowering=False, debug=False,
                   num_devices=NCORES)
    tabs = [nc.dram_tensor(f"t{q}", [TROWS, OUT_DIM], _f32,
                           kind="ExternalInput") for q in range(NT)]
    gidx = nc.dram_tensor("gidx", [P, meta["tot_idx_cols"]], _i16,
                          kind="ExternalInput")
    ldst = nc.dram_tensor("ldst", [P, meta["tot_chunks"]], _bf16,
                          kind="ExternalInput")
    ideg = nc.dram_tensor("ideg", [P, NG], _f32, kind="ExternalInput")
    brep = nc.dram_tensor("brep", [P, OUT_DIM], _f32, kind="ExternalInput")
    iota = nc.dram_tensor("iota", [P, P], _bf16, kind="ExternalInput")
    out = nc.dram_tensor("out", [G, OUT_DIM], _f32, kind="ExternalOutput")

    with tile.TileContext(nc) as tc:
        with (
            tc.tile_pool(name="const", bufs=1) as cpool,
            tc.tile_pool(name="gath", bufs=2) as gpool,
            tc.tile_pool(name="meta", bufs=2) as mpool,
            tc.tile_pool(name="onehot", bufs=4) as opool,
            tc.tile_pool(name="epi", bufs=4) as epool,
            tc.tile_pool(name="psum", bufs=8, space="PSUM") as psum,
        ):
            bt = cpool.tile([P, OUT_DIM], _f32, tag="b")
            it = cpool.tile([P, P], _bf16, tag="iota")
            nc.sync.dma_start(out=bt[:], in_=brep[:, :])
            nc.sync.dma_start(out=it[:], in_=iota[:, :])

            dt_ = cpool.tile([P, NG], _f32, tag="deg")
            norm = cpool.tile([P, NG], _f32, tag="norm")
            nc.sync.dma_start(out=dt_[:], in_=ideg[:, :])
            nc.vector.tensor_scalar_max(out=dt_[:], in0=dt_[:], scalar1=1.0)
            nc.vector.reciprocal(out=dt_[:], in_=dt_[:])
            nc.scalar.sqrt(out=norm[:], in_=dt_[:])

            for rnd in meta["rounds"]:
                nch = rnd["nch"]
                nidx_cols = sum(rnd["q_numidx"]) // 16
                ixt = mpool.tile([P, nidx_cols], _i16, tag="ix")
                nc.sync.dma_start(
                    out=ixt[:],
                    in_=gidx[:, rnd["idx_off"]:rnd["idx_off"] + nidx_cols])
                ldt = mpool.tile([P, nch], _bf16, tag="ld")
                nc.sync.dma_start(
                    out=ldt[:],
                    in_=ldst[:, rnd["ch_off"]:rnd["ch_off"] + nch])

                gt = gpool.tile([P, nch, OUT_DIM], _f32, tag="gt")
                icol = 0
                for q in range(NT):
                    nq = rnd["q_numidx"][q]
                    if nq == 0:
                        continue
                    co = rnd["q_choff"][q]
                    nc.gpsimd.dma_gather(
                        out_ap=gt[:, co:co + nq // P, :],
                        in_ap=tabs[q][:, :],
                        idxs_ap=ixt[:, icol:icol + nq // 16],
                        num_idxs=nq,
                        num_idxs_reg=nq,
                        elem_size=OUT_DIM,
                        single_packet=False,
                    )
                    icol += nq // 16

                # fp32 -> bf16 for full-rate PE
                gb = gpool.tile([P, nch, OUT_DIM], _bf16, tag="gb")
                nc.vector.tensor_copy(out=gb[:], in_=gt[:])

                for g in rnd["groups"]:
                    cols = rnd["gcols"][g]
                    x = epool.tile([P, OUT_DIM], _f32, tag="x")
                    if cols:
                        acc = psum.tile([P, OUT_DIM], _f32, tag="acc")
                        for k, col in enumerate(cols):
                            oh = opool.tile([P, P], _bf16, tag="oh")
                            nc.vector.tensor_tensor(
                                out=oh[:],
                                in0=ldt[:, col:col + 1].to_broadcast([P, P]),
                                in1=it[:],
                                op=mybir.AluOpType.is_equal)
                            nc.tensor.matmul(acc[:], oh[:], gb[:, col, :],
                                             start=(k == 0),
                                             stop=(k == len(cols) - 1))
                        # epilogue: out = agg*norm + b ; log_softmax
                        nc.scalar.activation(
                            out=x[:], in_=acc[:],
                            func=mybir.ActivationFunctionType.Identity,
                            scale=norm[:, g:g + 1])
                    else:
                        nc.vector.memset(x[:], 0.0)
                    nc.vector.tensor_add(out=x[:], in0=x[:], in1=bt[:])
                    nmx = epool.tile([P, 1], _f32, tag="nmx")
                    nc.vector.tensor_reduce(out=nmx[:], in_=x[:],
                                            axis=mybir.AxisListType.X,
                                            op=mybir.AluOpType.max,
                                            negate=True)
                    e = epool.tile([P, OUT_DIM], _f32, tag="e")
                    s = epool.tile([P, 1], _f32, tag="s")
                    nc.scalar.activation(
                        out=e[:], in_=x[:],
                        func=mybir.ActivationFunctionType.Exp,
                        bias=nmx[:, :1], accum_out=s[:, :1])
                    ls = epool.tile([P, 1], _f32, tag="ls")
                    nc.scalar.activation(
                        out=ls[:], in_=s[:],
                        func=mybir.ActivationFunctionType.Ln)
                    adj = epool.tile([P, 1], _f32, tag="adj")
                    nc.vector.tensor_tensor(out=adj[:], in0=nmx[:], in1=ls[:],
                                            op=mybir.AluOpType.subtract)
                    fin = epool.tile([P, OUT_DIM], _f32, tag="fin")
                    nc.vector.tensor_scalar_add(out=fin[:], in0=x[:],
                                                scalar1=adj[:, :1])
                    rows = min(P, G - g * P)
                    nc.sync.dma_start(out=out[g * P:g * P + rows, :],
                                      in_=fin[:rows, :])
    nc.compile()
    return nc


# ------------------------------------------------------------- host prep
def _wrap_idx16(flat):
    """int16 index list -> [128, len/16] wrapped layout (16-partition groups,
    replicated across the 8 gpsimd cores)."""
    n = len(flat)
    s = n // 16
    arr = np.empty((P, s), dtype=np.int16)
    blk = flat.reshape(s, 16).T  # [16, s]
    for grp in range(8):
        arr[grp * 16:(grp + 1) * 16, :] = blk
    return arr


def prepare(h, W, b, edges):
    h = np.asarray(h, dtype=np.float32)
    W = np.asarray(W, dtype=np.float32)
    b = np.asarray(b, dtype=np.float32)
    src = np.asarray(edges[0], dtype=np.int64)
    dst = np.asarray(edges[1], dtype=np.int64)

    out_deg = np.bincount(src, minlength=N_NODES).astype(np.float32)
    in_deg = np.bincount(dst, minlength=N_NODES).astype(np.float32)

    # global m-table row for each src node (padded per-core layout)
    score = src // G
    mrow = score * GPAD + (src - score * G)
    qtab = mrow // TROWS
    lrow = (mrow - qtab * TROWS).astype(np.int16)

    dcore = dst // G
    dloc = dst - dcore * G
    grp = dloc // P
    ldst_v = (dloc - grp * P).astype(np.float32)

    # bucket = (dst-core, group, sub-table)
    bucket = (dcore * NG + grp) * NT + qtab
    order = np.argsort(bucket, kind="stable")
    bucket_s = bucket[order]
    lrow_s = lrow[order]
    ldst_s = ldst_v[order]

    nbuck = NCORES * NG * NT
    counts = np.bincount(bucket_s, minlength=nbuck).reshape(NCORES, NG, NT)
    starts = np.zeros(nbuck + 1, dtype=np.int64)
    np.cumsum(counts.reshape(-1), out=starts[1:])

    # uniform capacity per (group, sub-table): max over cores, ceil to 128
    cap = counts.max(axis=0)                      # [NG, NT]
    cap128 = ((cap + P - 1) // P) * P             # [NG, NT]

    # round structure (uniform across cores)
    rounds = []
    idx_off = 0
    ch_off = 0
    for r0 in range(0, NG, ROUND_G):
        gs = list(range(r0, min(r0 + ROUND_G, NG)))
        q_numidx, q_choff, gcols = [], [], {g: [] for g in gs}
        cursor = 0
        for q in range(NT):
            q_choff.append(cursor)
            tot = 0
            for g in gs:
                c = int(cap128[g, q])
                gcols[g].extend(range(cursor, cursor + c // P))
                cursor += c // P
                tot += c
            q_numidx.append(tot)
        rounds.append(dict(groups=gs, q_numidx=q_numidx, q_choff=q_choff,
                           nch=cursor, idx_off=idx_off, ch_off=ch_off,
                           gcols=gcols))
        idx_off += sum(q_numidx) // 16
        ch_off += cursor
    meta = dict(rounds=rounds, tot_idx_cols=idx_off, tot_chunks=ch_off)

    # per-core gidx / ldst arrays
    gidx_cores = []
    ldst_cores = []
    for c in range(NCORES):
        flat_idx = np.zeros(idx_off * 16, dtype=np.int16)
        ld = np.full((P, ch_off), PAD_LDST, dtype=np.float32)
        for rnd in rounds:
            pos = rnd["idx_off"] * 16
            for q in range(NT):
                for g in rnd["groups"]:
                    bid = (c * NG + g) * NT + q
                    s0, s1 = starts[bid], starts[bid + 1]
                    n = s1 - s0
                    capq = int(cap128[g, q])
                    # indices: real then pad-with-0
                    flat_idx[pos:pos + n] = lrow_s[s0:s1]
                    pos += capq
                    # ldst columns for this bucket
                    base = rnd["ch_off"] + rnd["q_choff"][q] + \
                        sum(int(cap128[g2, q]) // P
                            for g2 in rnd["groups"] if g2 < g)
                    j = np.arange(n)
                    ld[j % P, base + j // P] = ldst_s[s0:s1]
        gidx_cores.append(_wrap_idx16(flat_idx))
        ldst_cores.append(ld.astype(ml_dtypes.bfloat16))

    # degree tiles [128, NG] (partition = node % 128 within group)
    def deg_tile(deg):
        tiles = []
        for c in range(NCORES):
            d = np.ones(GPAD, dtype=np.float32)
            d[:G] = deg[c * G:(c + 1) * G]
            tiles.append(d.reshape(NG, P).T.copy())
        return tiles

    odeg_tiles = deg_tile(out_deg)
    ideg_tiles = deg_tile(in_deg)

    hT_cores = []
    for c in range(NCORES):
        hp = np.zeros((GPAD, IN_DIM), dtype=np.float32)
        hp[:G] = h[c * G:(c + 1) * G]
        # [NG, IN_DIM, P]: per-group contiguous blocks for single-DMA loads
        hT_cores.append(
            np.ascontiguousarray(hp.reshape(NG, P, IN_DIM).transpose(0, 2, 1)))

    brep = np.broadcast_to(b, (P, OUT_DIM)).copy()
    iota = np.broadcast_to(np.arange(P, dtype=np.float32), (P, P)) \
        .astype(ml_dtypes.bfloat16).copy()

    return dict(meta=meta, gidx=gidx_cores, ldst=ldst_cores,
                odeg=odeg_tiles, ideg=ideg_tiles, hT=hT_cores,
                W=W, brep=brep, iota=iota)


_cache = {}


def _get_programs(meta):
    if "a" not in _cache:
        _cache["a"] = build_launch_a()
    if "b" not in _cache:
        _cache["b"] = build_launch_b(meta)
    return _cache["a"], _cache["b"]


def run_launch_a(nc_a, prep):
    in_maps = [{"hT": prep["hT"][c], "W": prep["W"], "odeg": prep["odeg"][c]}
               for c in range(NCORES)]
    res = run_bass_kernel_spmd(nc_a, in_maps, list(range(NCORES)))
    return [r["m"] for r in res.results]


def run_launch_b(nc_b, prep, m_shards):
    m_full = np.concatenate(m_shards, axis=0)  # [NCORES*GPAD, 64]
    tabs = {f"t{q}": np.ascontiguousarray(m_full[q * TROWS:(q + 1) * TROWS])
            for q in range(NT)}
    in_maps = [dict(tabs, gidx=prep["gidx"][c], ldst=prep["ldst"][c],
                    ideg=prep["ideg"][c], brep=prep["brep"],
                    iota=prep["iota"]) for c in range(NCORES)]
    res = run_bass_kernel_spmd(nc_b, in_maps, list(range(NCORES)))
    return np.concatenate([r["out"] for r in res.results], axis=0)


def kernel(h, W, b, edges):
    prep = prepare(h, W, b, edges)
    nc_a, nc_b = _get_programs(prep["meta"])
    m_shards = run_launch_a(nc_a, prep)
    out = run_launch_b(nc_b, prep, m_shards)
    return out.astype(np.float32)


# revision 26
# speedup vs baseline: 8714.8821x; 8714.8821x over previous
"""GraphConv (DGL norm='both') + log_softmax on 8 Trainium2 NeuronCores.

Strategy (per sharding hint): partition nodes across the 8 cores by range.
  Launch A (per core): project its 12500-node slice m = (h @ W) * out_deg^-1/2.
  Host: concatenate the 8 projected shards into a replicated gather table.
  Launch B (per core): for its 12500 dst nodes, gather m[src] rows for all
  in-edges (dma_gather, edges pre-sorted by dst group), segment-sum via
  one-hot matmuls accumulating in PSUM, then norm/bias/log_softmax.

Degrees and the sorted/padded edge metadata are sharding-prep computed on the
host (numpy); all FLOPs on h/W/b/m (projection, normalization, aggregation,
softmax) run on device.
"""

import numpy as np
import ml_dtypes

import concourse.bass as bass
import concourse.bacc as bacc
import concourse.mybir as mybir
import concourse.tile as tile
from concourse.bass import AP
from concourse.bass_utils import run_bass_kernel_spmd

P = 128
N_NODES = 100000
N_EDGES = 3200000
IN_DIM = 256
OUT_DIM = 64
NCORES = 8
G = N_NODES // NCORES            # 12500 nodes per core
NG = (G + P - 1) // P            # 98 groups of 128 dst nodes (last has 84)
GPAD = NG * P                    # 12544
NT = 4                           # gather sub-tables (int16 index limit)
TROWS = (NCORES * GPAD) // NT    # 25088 rows per sub-table
ROUND_G = 4                      # dst groups per gather round
HBLK = 8                         # dst groups per hT load in launch A
PAD_LDST = 200.0                 # local-dst for padded edges (>127, exact bf16)

_f32 = mybir.dt.float32
_bf16 = mybir.dt.bfloat16
_i16 = mybir.dt.int16


def _expand_mid(ap, n):
    """[P, C] AP -> [P, n, C] AP repeating each partition row n times
    (middle broadcast keeps the last dim packed, so DVE 2x mode applies)."""
    (ps, pc), (cs, cc) = ap.ap[0], ap.ap[1]
    return AP(ap.tensor, ap.offset, [[ps, pc], [0, n], [cs, cc]])


import contextlib


# ---------------------------------------------------------------- launch A
def build_launch_a(repeat=1):
    nc = bacc.Bacc("TRN2", target_bir_lowering=False, debug=False,
                   num_devices=NCORES)
    hT = nc.dram_tensor("hT", [2, P, GPAD], _f32, kind="ExternalInput")
    W = nc.dram_tensor("W", [IN_DIM, OUT_DIM], _f32, kind="ExternalInput")
    odeg = nc.dram_tensor("odeg", [P, NG], _f32, kind="ExternalInput")
    m = nc.dram_tensor("m", [GPAD, OUT_DIM], _f32, kind="ExternalOutput")

    with tile.TileContext(nc) as tc:
        loop = tc.For_i(0, repeat, 1) if repeat > 1 \
            else contextlib.nullcontext()
        with loop, \
                tc.tile_pool(name="const", bufs=1) as cpool, \
                tc.tile_pool(name="hblk", bufs=3) as hpool, \
                tc.tile_pool(name="work", bufs=4) as pool, \
                tc.tile_pool(name="psum", bufs=4, space="PSUM") as psum:
            w0 = cpool.tile([P, OUT_DIM], _f32, tag="w0")
            w1 = cpool.tile([P, OUT_DIM], _f32, tag="w1")
            nc.sync.dma_start(out=w0[:], in_=W[0:P, :])
            nc.sync.dma_start(out=w1[:], in_=W[P:2 * P, :])

            dt_ = cpool.tile([P, NG], _f32, tag="deg")
            norm = cpool.tile([P, NG], _f32, tag="norm")
            nc.sync.dma_start(out=dt_[:], in_=odeg[:, :])
            nc.vector.tensor_scalar_max(out=dt_[:], in0=dt_[:], scalar1=1.0)
            nc.vector.reciprocal(out=dt_[:], in_=dt_[:])
            nc.scalar.sqrt(out=norm[:], in_=dt_[:])

            for g0 in range(0, NG, HBLK):
                nb = min(HBLK, NG - g0)
                l0 = hpool.tile([P, HBLK * P], _f32, tag="l0")
                l1 = hpool.tile([P, HBLK * P], _f32, tag="l1")
                nc.sync.dma_start(out=l0[:, :nb * P],
                                  in_=hT[0, :, g0 * P:(g0 + nb) * P])
                nc.sync.dma_start(out=l1[:, :nb * P],
                                  in_=hT[1, :, g0 * P:(g0 + nb) * P])
                for j in range(nb):
                    g = g0 + j
                    acc = psum.tile([P, OUT_DIM], _f32, tag="acc")
                    nc.tensor.matmul(acc[:], l0[:, j * P:(j + 1) * P], w0[:],
                                     start=True, stop=False)
                    nc.tensor.matmul(acc[:], l1[:, j * P:(j + 1) * P], w1[:],
                                     start=False, stop=True)
                    ms = pool.tile([P, OUT_DIM], _f32, tag="ms")
                    nc.scalar.activation(
                        out=ms[:], in_=acc[:],
                        func=mybir.ActivationFunctionType.Identity,
                        scale=norm[:, g:g + 1])
                    nc.sync.dma_start(out=m[g * P:(g + 1) * P, :], in_=ms[:])
    nc.compile()
    return nc


# ---------------------------------------------------------------- launch B
def build_launch_b(meta, repeat=1):
    """meta["rounds"][i]:
      groups; q_numidx[NT]; q_choff[NT]; nch; idx_off; ch_off
      gldt: {g: (ldt_col_start, ngch)}   # ldst cols, group-major contiguous
      ggt:  {g: [gt_column, ...]}        # gather-tile column per oh chunk
    """
    nc = bacc.Bacc("TRN2", target_bir_lowering=False, debug=False,
                   num_devices=NCORES)
    tabs = [nc.dram_tensor(f"t{q}", [TROWS, OUT_DIM], _f32,
                           kind="ExternalInput") for q in range(NT)]
    gidx = nc.dram_tensor("gidx", [P, meta["tot_idx_cols"]], _i16,
                          kind="ExternalInput")
    ldst = nc.dram_tensor("ldst", [P, meta["tot_chunks"]], _bf16,
                          kind="ExternalInput")
    max_gch = meta["max_gch"]
    ideg = nc.dram_tensor("ideg", [P, NG], _f32, kind="ExternalInput")
    brep = nc.dram_tensor("brep", [P, OUT_DIM], _f32, kind="ExternalInput")
    # iotar[p, m, c] = m  (bf16) — constant compare target for one-hot builds
    iotar = nc.dram_tensor("iotar", [P, P, max_gch], _bf16,
                           kind="ExternalInput")
    out = nc.dram_tensor("out", [NG, P, OUT_DIM], _f32, kind="ExternalOutput")

    with tile.TileContext(nc) as tc:
        loop = tc.For_i(0, repeat, 1) if repeat > 1 \
            else contextlib.nullcontext()
        with loop, \
                tc.tile_pool(name="const", bufs=1) as cpool, \
                tc.tile_pool(name="gath", bufs=2) as gpool, \
                tc.tile_pool(name="conv", bufs=2) as vpool, \
                tc.tile_pool(name="meta", bufs=2) as mpool, \
                tc.tile_pool(name="onehot", bufs=3) as opool, \
                tc.tile_pool(name="epi", bufs=4) as epool, \
                tc.tile_pool(name="psum", bufs=8, space="PSUM") as psum:
            bt = cpool.tile([P, OUT_DIM], _f32, tag="b")
            it = cpool.tile([P, P, max_gch], _bf16, tag="iotar")
            nc.sync.dma_start(out=bt[:], in_=brep[:, :])
            nc.sync.dma_start(out=it[:], in_=iotar[:, :, :])

            dt_ = cpool.tile([P, NG], _f32, tag="deg")
            norm = cpool.tile([P, NG], _f32, tag="norm")
            nc.sync.dma_start(out=dt_[:], in_=ideg[:, :])
            nc.vector.tensor_scalar_max(out=dt_[:], in0=dt_[:], scalar1=1.0)
            nc.vector.reciprocal(out=dt_[:], in_=dt_[:])
            nc.scalar.sqrt(out=norm[:], in_=dt_[:])

            # persistent per-group staging: y = x - max(x); s = sum(exp)
            y_all = cpool.tile([P, NG * OUT_DIM], _f32, tag="yall")
            s_all = cpool.tile([P, NG], _f32, tag="sall")

            for ri, rnd in enumerate(meta["rounds"]):
                nch = rnd["nch"]
                nidx_cols = sum(rnd["q_numidx"]) // 16
                ixt = mpool.tile([P, nidx_cols], _i16, tag="ix")
                nc.sync.dma_start(
                    out=ixt[:],
                    in_=gidx[:, rnd["idx_off"]:rnd["idx_off"] + nidx_cols])
                ldt = mpool.tile([P, nch], _bf16, tag="ld")
                nc.sync.dma_start(
                    out=ldt[:],
                    in_=ldst[:, rnd["ch_off"]:rnd["ch_off"] + nch])

                gt = gpool.tile([P, nch, OUT_DIM], _f32, tag="gt")
                if ri < 2:
                    # first use of each rotating slot (bufs=2): clear so chunk
                    # padding never feeds NaN bit-patterns into the matmul
                    nc.gpsimd.memset(gt[:], 0.0)
                icol = 0
                for q in range(NT):
                    nq = rnd["q_numidx"][q]
                    if nq == 0:
                        continue
                    co = rnd["q_choff"][q]
                    nc.gpsimd.dma_gather(
                        out_ap=gt[:, co:co + nq // P, :],
                        in_ap=tabs[q][:, :],
                        idxs_ap=ixt[:, icol:icol + nq // 16],
                        num_idxs=nq,
                        num_idxs_reg=nq,
                        elem_size=OUT_DIM,
                        single_packet=False,
                    )
                    icol += nq // 16

                # fp32 -> bf16 rhs for full-rate PE; alternate engines by round
                gb = vpool.tile([P, nch, OUT_DIM], _bf16, tag="gb")
                if ri % 2 == 0:
                    nc.vector.tensor_copy(out=gb[:], in_=gt[:])
                else:
                    nc.scalar.activation(
                        out=gb[:], in_=gt[:],
                        func=mybir.ActivationFunctionType.Copy)

                for g in rnd["groups"]:
                    ldt0, ngch = rnd["gldt"][g]
                    gtcols = rnd["ggt"][g]
                    x = epool.tile([P, OUT_DIM], _f32, tag="x")
                    if ngch:
                        # one-hot, chunk-last: oh[k, m, c] = (ldst[k,col_c]==m)
                        # all APs keep a packed last dim -> DVE 2x mode
                        oh = opool.tile([P, P, max_gch], _bf16, tag="oh")
                        nc.vector.tensor_tensor(
                            out=oh[:, :, 0:ngch],
                            in0=_expand_mid(ldt[:, ldt0:ldt0 + ngch], P),
                            in1=it[:, :, 0:ngch],
                            op=mybir.AluOpType.is_equal)
                        acc = psum.tile([P, OUT_DIM], _f32, tag="acc")
                        for k, cg in enumerate(gtcols):
                            nc.tensor.matmul(
                                acc[:], oh[:, :, k], gb[:, cg, :],
                                start=(k == 0), stop=(k == ngch - 1))
                        nc.scalar.activation(
                            out=x[:], in_=acc[:],
                            func=mybir.ActivationFunctionType.Identity,
                            scale=norm[:, g:g + 1])
                    else:
                        nc.vector.memset(x[:], 0.0)
                    nc.vector.tensor_add(out=x[:], in0=x[:], in1=bt[:])
                    nmx = epool.tile([P, 1], _f32, tag="nmx")
                    nc.vector.tensor_reduce(out=nmx[:], in_=x[:],
                                            axis=mybir.AxisListType.X,
                                            op=mybir.AluOpType.max,
                                            negate=True)
                    nc.vector.tensor_scalar_add(
                        out=y_all[:, g * OUT_DIM:(g + 1) * OUT_DIM],
                        in0=x[:], scalar1=nmx[:, :1])
                    e = epool.tile([P, OUT_DIM], _f32, tag="e")
                    nc.scalar.activation(
                        out=e[:], in_=x[:],
                        func=mybir.ActivationFunctionType.Exp,
                        bias=nmx[:, :1], accum_out=s_all[:, g:g + 1])

            # single Ln over all groups, then finalize + store per group
            ls_all = cpool.tile([P, NG], _f32, tag="lsall")
            nc.scalar.activation(out=ls_all[:], in_=s_all[:],
                                 func=mybir.ActivationFunctionType.Ln)
            for g in range(NG):
                fin = epool.tile([P, OUT_DIM], _f32, tag="fin")
                nc.vector.tensor_scalar_sub(
                    out=fin[:],
                    in0=y_all[:, g * OUT_DIM:(g + 1) * OUT_DIM],
                    scalar1=ls_all[:, g:g + 1])
                rows = min(P, G - g * P)
                nc.sync.dma_start(out=out[g, :rows, :], in_=fin[:rows, :])
    nc.compile()
    return nc


# ------------------------------------------------------------- host prep
def _wrap_idx16(flat):
    """int16 index list -> [128, len/16] wrapped layout (16-partition groups,
    replicated across the 8 gpsimd cores)."""
    n = len(flat)
    s = n // 16
    arr = np.empty((P, s), dtype=np.int16)
    blk = flat.reshape(s, 16).T  # [16, s]
    for grp in range(8):
        arr[grp * 16:(grp + 1) * 16, :] = blk
    return arr


def prepare(h, W, b, edges):
    h = np.asarray(h, dtype=np.float32)
    W = np.asarray(W, dtype=np.float32)
    b = np.asarray(b, dtype=np.float32)
    src = np.asarray(edges[0], dtype=np.int64)
    dst = np.asarray(edges[1], dtype=np.int64)

    out_deg = np.bincount(src, minlength=N_NODES).astype(np.float32)
    in_deg = np.bincount(dst, minlength=N_NODES).astype(np.float32)

    # global m-table row for each src node (padded per-core layout)
    score = src // G
    mrow = score * GPAD + (src - score * G)
    qtab = mrow // TROWS
    lrow = (mrow - qtab * TROWS).astype(np.int16)

    dcore = dst // G
    dloc = dst - dcore * G
    grp = dloc // P
    ldst_v = (dloc - grp * P).astype(np.float32)

    # bucket = (dst-core, group, sub-table)
    bucket = (dcore * NG + grp) * NT + qtab
    order = np.argsort(bucket, kind="stable")
    bucket_s = bucket[order]
    lrow_s = lrow[order]
    ldst_s = ldst_v[order]

    nbuck = NCORES * NG * NT
    counts = np.bincount(bucket_s, minlength=nbuck).reshape(NCORES, NG, NT)
    starts = np.zeros(nbuck + 1, dtype=np.int64)
    np.cumsum(counts.reshape(-1), out=starts[1:])

    # uniform capacity per (group, sub-table): max over cores, ceil to 128
    cap = counts.max(axis=0)                      # [NG, NT]
    cap128 = ((cap + P - 1) // P) * P             # [NG, NT]

    # round structure (uniform across cores)
    # gather tile columns: (q, g, chunk) order; ldst columns: (g, q, chunk)
    rounds = []
    idx_off = 0
    ch_off = 0
    for r0 in range(0, NG, ROUND_G):
        gs = list(range(r0, min(r0 + ROUND_G, NG)))
        q_numidx, q_choff = [], []
        gt_col = {}          # (g, q) -> gather-tile column base (within round)
        cursor = 0
        for q in range(NT):
            q_choff.append(cursor)
            tot = 0
            for g in gs:
                c = int(cap128[g, q])
                gt_col[(g, q)] = cursor
                cursor += c // P
                tot += c
            q_numidx.append(tot)
        gldt, ggt = {}, {}
        lcur = 0
        for g in gs:
            cols = []
            for q in range(NT):
                cols.extend(range(gt_col[(g, q)],
                                  gt_col[(g, q)] + int(cap128[g, q]) // P))
            gldt[g] = (lcur, len(cols))
            ggt[g] = cols
            lcur += len(cols)
        rounds.append(dict(groups=gs, q_numidx=q_numidx, q_choff=q_choff,
                           nch=cursor, idx_off=idx_off, ch_off=ch_off,
                           gldt=gldt, ggt=ggt, gt_col=gt_col))
        idx_off += sum(q_numidx) // 16
        ch_off += cursor
    max_gch = max(rnd["gldt"][g][1] for rnd in rounds for g in rnd["groups"])
    meta = dict(rounds=rounds, tot_idx_cols=idx_off, tot_chunks=ch_off,
                max_gch=max_gch)

    # per-core gidx / ldst arrays
    gidx_cores = []
    ldst_cores = []
    for c in range(NCORES):
        flat_idx = np.zeros(idx_off * 16, dtype=np.int16)
        ld = np.full((P, ch_off), PAD_LDST, dtype=np.float32)
        for rnd in rounds:
            pos = rnd["idx_off"] * 16
            for q in range(NT):
                for g in rnd["groups"]:
                    bid = (c * NG + g) * NT + q
                    s0, s1 = starts[bid], starts[bid + 1]
                    n = s1 - s0
                    capq = int(cap128[g, q])
                    flat_idx[pos:pos + n] = lrow_s[s0:s1]
                    pos += capq
                    # ldst column base: group-major layout
                    qch0 = sum(int(cap128[g, q2]) // P for q2 in range(q))
                    base = rnd["ch_off"] + rnd["gldt"][g][0] + qch0
                    j = np.arange(n)
                    ld[j % P, base + j // P] = ldst_s[s0:s1]
        gidx_cores.append(_wrap_idx16(flat_idx))
        ldst_cores.append(ld.astype(ml_dtypes.bfloat16))

    # degree tiles [128, NG] (partition = node % 128 within group)
    def deg_tile(deg):
        tiles = []
        for c in range(NCORES):
            d = np.ones(GPAD, dtype=np.float32)
            d[:G] = deg[c * G:(c + 1) * G]
            tiles.append(d.reshape(NG, P).T.copy())
        return tiles

    odeg_tiles = deg_tile(out_deg)
    ideg_tiles = deg_tile(in_deg)

    hT_cores = []
    for c in range(NCORES):
        hp = np.zeros((GPAD, IN_DIM), dtype=np.float32)
        hp[:G] = h[c * G:(c + 1) * G]
        # [2, 128, GPAD]: k-halves, contiguous along nodes for wide DMAs
        ht = np.ascontiguousarray(hp.T.reshape(2, P, GPAD))
        hT_cores.append(ht)

    brep = np.broadcast_to(b, (P, OUT_DIM)).copy()
    # iotar[p, m, c] = m
    iotar = np.broadcast_to(
        np.arange(P, dtype=np.float32)[None, :, None],
        (P, P, max_gch)).astype(ml_dtypes.bfloat16).copy()

    return dict(meta=meta, gidx=gidx_cores, ldst=ldst_cores,
                odeg=odeg_tiles, ideg=ideg_tiles, hT=hT_cores,
                W=W, brep=brep, iotar=iotar)


_cache = {}


def _get_programs(meta):
    if "a" not in _cache:
        _cache["a"] = build_launch_a()
    if "b" not in _cache:
        _cache["b"] = build_launch_b(meta)
    return _cache["a"], _cache["b"]


def run_launch_a(nc_a, prep):
    in_maps = [{"hT": prep["hT"][c], "W": prep["W"], "odeg": prep["odeg"][c]}
               for c in range(NCORES)]
    res = run_bass_kernel_spmd(nc_a, in_maps, list(range(NCORES)))
    return [r["m"] for r in res.results]


def run_launch_b(nc_b, prep, m_shards):
    m_full = np.concatenate(m_shards, axis=0)  # [NCORES*GPAD, 64]
    tabs = {f"t{q}": np.ascontiguousarray(m_full[q * TROWS:(q + 1) * TROWS])
            for q in range(NT)}
    in_maps = [dict(tabs, gidx=prep["gidx"][c], ldst=prep["ldst"][c],
                    ideg=prep["ideg"][c], brep=prep["brep"],
                    iotar=prep["iotar"]) for c in range(NCORES)]
    res = run_bass_kernel_spmd(nc_b, in_maps, list(range(NCORES)))
    return np.concatenate(
        [r["out"].reshape(GPAD, OUT_DIM)[:G] for r in res.results], axis=0)


def kernel(h, W, b, edges):
    prep = prepare(h, W, b, edges)
    nc_a, nc_b = _get_programs(prep["meta"])
    m_shards = run_launch_a(nc_a, prep)
    out = run_launch_b(nc_b, prep, m_shards)
    return out.astype(np.float32)


# revision 29
# speedup vs baseline: 9211.3907x; 1.0570x over previous
"""GraphConv (DGL norm='both') + log_softmax on 8 Trainium2 NeuronCores.

Strategy (per sharding hint): partition nodes across the 8 cores by range.
  Launch A (per core): project its 12500-node slice m = (h @ W) * out_deg^-1/2.
  Host: concatenate the 8 projected shards into a replicated gather table.
  Launch B (per core): for its 12500 dst nodes, gather m[src] rows for all
  in-edges (dma_gather, edges pre-sorted by dst group), segment-sum via
  one-hot matmuls accumulating in PSUM, then norm/bias/log_softmax.

Degrees and the sorted/padded edge metadata are sharding-prep computed on the
host (numpy); all FLOPs on h/W/b/m (projection, normalization, aggregation,
softmax) run on device.
"""

import numpy as np
import ml_dtypes

import concourse.bass as bass
import concourse.bacc as bacc
import concourse.mybir as mybir
import concourse.tile as tile
from concourse.bass import AP
from concourse.bass_utils import run_bass_kernel_spmd

P = 128
N_NODES = 100000
N_EDGES = 3200000
IN_DIM = 256
OUT_DIM = 64
NCORES = 8
G = N_NODES // NCORES            # 12500 nodes per core
NG = (G + P - 1) // P            # 98 groups of 128 dst nodes (last has 84)
GPAD = NG * P                    # 12544
NT = 4                           # gather sub-tables (int16 index limit)
TROWS = (NCORES * GPAD) // NT    # 25088 rows per sub-table
ROUND_G = 4                      # dst groups per gather round
HBLK = 8                         # dst groups per hT load in launch A
PAD_LDST = 200.0                 # local-dst for padded edges (>127, exact bf16)

_f32 = mybir.dt.float32
_bf16 = mybir.dt.bfloat16
_i16 = mybir.dt.int16


def _expand_mid(ap, n):
    """[P, C] AP -> [P, n, C] AP repeating each partition row n times
    (middle broadcast keeps the last dim packed, so DVE 2x mode applies)."""
    (ps, pc), (cs, cc) = ap.ap[0], ap.ap[1]
    return AP(ap.tensor, ap.offset, [[ps, pc], [0, n], [cs, cc]])


import contextlib


# ---------------------------------------------------------------- launch A
def build_launch_a(repeat=1):
    nc = bacc.Bacc("TRN2", target_bir_lowering=False, debug=False,
                   num_devices=NCORES)
    hT = nc.dram_tensor("hT", [2, P, GPAD], _f32, kind="ExternalInput")
    W = nc.dram_tensor("W", [IN_DIM, OUT_DIM], _f32, kind="ExternalInput")
    odeg = nc.dram_tensor("odeg", [P, NG], _f32, kind="ExternalInput")
    m = nc.dram_tensor("m", [GPAD, OUT_DIM], _f32, kind="ExternalOutput")

    with tile.TileContext(nc) as tc:
        loop = tc.For_i(0, repeat, 1) if repeat > 1 \
            else contextlib.nullcontext()
        with loop, \
                tc.tile_pool(name="const", bufs=1) as cpool, \
                tc.tile_pool(name="hblk", bufs=3) as hpool, \
                tc.tile_pool(name="work", bufs=4) as pool, \
                tc.tile_pool(name="psum", bufs=4, space="PSUM") as psum:
            w0 = cpool.tile([P, OUT_DIM], _f32, tag="w0")
            w1 = cpool.tile([P, OUT_DIM], _f32, tag="w1")
            nc.sync.dma_start(out=w0[:], in_=W[0:P, :])
            nc.sync.dma_start(out=w1[:], in_=W[P:2 * P, :])

            dt_ = cpool.tile([P, NG], _f32, tag="deg")
            norm = cpool.tile([P, NG], _f32, tag="norm")
            nc.sync.dma_start(out=dt_[:], in_=odeg[:, :])
            nc.vector.tensor_scalar_max(out=dt_[:], in0=dt_[:], scalar1=1.0)
            nc.vector.reciprocal(out=dt_[:], in_=dt_[:])
            nc.scalar.sqrt(out=norm[:], in_=dt_[:])

            for g0 in range(0, NG, HBLK):
                nb = min(HBLK, NG - g0)
                l0 = hpool.tile([P, HBLK * P], _f32, tag="l0")
                l1 = hpool.tile([P, HBLK * P], _f32, tag="l1")
                nc.sync.dma_start(out=l0[:, :nb * P],
                                  in_=hT[0, :, g0 * P:(g0 + nb) * P])
                nc.sync.dma_start(out=l1[:, :nb * P],
                                  in_=hT[1, :, g0 * P:(g0 + nb) * P])
                for j in range(nb):
                    g = g0 + j
                    acc = psum.tile([P, OUT_DIM], _f32, tag="acc")
                    nc.tensor.matmul(acc[:], l0[:, j * P:(j + 1) * P], w0[:],
                                     start=True, stop=False)
                    nc.tensor.matmul(acc[:], l1[:, j * P:(j + 1) * P], w1[:],
                                     start=False, stop=True)
                    ms = pool.tile([P, OUT_DIM], _f32, tag="ms")
                    nc.scalar.activation(
                        out=ms[:], in_=acc[:],
                        func=mybir.ActivationFunctionType.Identity,
                        scale=norm[:, g:g + 1])
                    nc.sync.dma_start(out=m[g * P:(g + 1) * P, :], in_=ms[:])
    nc.compile()
    return nc


# ---------------------------------------------------------------- launch B
def build_launch_b(meta, repeat=1):
    """meta["rounds"][i]:
      groups; q_numidx[NT]; q_choff[NT]; nch; idx_off; ch_off
      gldt: {g: (ldt_col_start, ngch)}   # ldst cols, group-major contiguous
      ggt:  {g: [gt_column, ...]}        # gather-tile column per oh chunk
    """
    nc = bacc.Bacc("TRN2", target_bir_lowering=False, debug=False,
                   num_devices=NCORES)
    tabs = [nc.dram_tensor(f"t{q}", [TROWS, OUT_DIM], _f32,
                           kind="ExternalInput") for q in range(NT)]
    gidx = nc.dram_tensor("gidx", [P, meta["tot_idx_cols"]], _i16,
                          kind="ExternalInput")
    ldst = nc.dram_tensor("ldst", [P, meta["tot_chunks"]], _bf16,
                          kind="ExternalInput")
    max_gch = meta["max_gch"]
    ideg = nc.dram_tensor("ideg", [P, NG], _f32, kind="ExternalInput")
    brep = nc.dram_tensor("brep", [P, OUT_DIM], _f32, kind="ExternalInput")
    # iotar[p, m, c] = m  (bf16) — constant compare target for one-hot builds
    iotar = nc.dram_tensor("iotar", [P, P, max_gch], _bf16,
                           kind="ExternalInput")
    out = nc.dram_tensor("out", [NG, P, OUT_DIM], _f32, kind="ExternalOutput")

    with tile.TileContext(nc) as tc:
        loop = tc.For_i(0, repeat, 1) if repeat > 1 \
            else contextlib.nullcontext()
        with loop, \
                tc.tile_pool(name="const", bufs=1) as cpool, \
                tc.tile_pool(name="gath", bufs=2) as gpool, \
                tc.tile_pool(name="conv", bufs=2) as vpool, \
                tc.tile_pool(name="meta", bufs=2) as mpool, \
                tc.tile_pool(name="onehot", bufs=3) as opool, \
                tc.tile_pool(name="epi", bufs=4) as epool, \
                tc.tile_pool(name="psum", bufs=8, space="PSUM") as psum:
            bt = cpool.tile([P, OUT_DIM], _f32, tag="b")
            it = cpool.tile([P, P, max_gch], _bf16, tag="iotar")
            nc.sync.dma_start(out=bt[:], in_=brep[:, :])
            nc.sync.dma_start(out=it[:], in_=iotar[:, :, :])

            dt_ = cpool.tile([P, NG], _f32, tag="deg")
            norm = cpool.tile([P, NG], _f32, tag="norm")
            nc.sync.dma_start(out=dt_[:], in_=ideg[:, :])
            nc.vector.tensor_scalar_max(out=dt_[:], in0=dt_[:], scalar1=1.0)
            nc.vector.reciprocal(out=dt_[:], in_=dt_[:])
            nc.scalar.sqrt(out=norm[:], in_=dt_[:])

            # persistent per-group staging: y = x - max(x); s = sum(exp)
            y_all = cpool.tile([P, NG * OUT_DIM], _f32, tag="yall")
            s_all = cpool.tile([P, NG], _f32, tag="sall")

            for ri, rnd in enumerate(meta["rounds"]):
                nch = rnd["nch"]
                nidx_cols = sum(rnd["q_numidx"]) // 16
                ixt = mpool.tile([P, nidx_cols], _i16, tag="ix")
                nc.sync.dma_start(
                    out=ixt[:],
                    in_=gidx[:, rnd["idx_off"]:rnd["idx_off"] + nidx_cols])
                ldt = mpool.tile([P, nch], _bf16, tag="ld")
                nc.sync.dma_start(
                    out=ldt[:],
                    in_=ldst[:, rnd["ch_off"]:rnd["ch_off"] + nch])

                gt = gpool.tile([P, nch, OUT_DIM], _f32, tag="gt")
                if ri < 2:
                    # first use of each rotating slot (bufs=2): clear so chunk
                    # padding never feeds NaN bit-patterns into the matmul
                    nc.gpsimd.memset(gt[:], 0.0)
                icol = 0
                for q in range(NT):
                    nq = rnd["q_numidx"][q]
                    if nq == 0:
                        continue
                    co = rnd["q_choff"][q]
                    nc.gpsimd.dma_gather(
                        out_ap=gt[:, co:co + nq // P, :],
                        in_ap=tabs[q][:, :],
                        idxs_ap=ixt[:, icol:icol + nq // 16],
                        num_idxs=nq,
                        num_idxs_reg=nq,
                        elem_size=OUT_DIM,
                        single_packet=False,
                    )
                    icol += nq // 16

                # fp32 -> bf16 rhs for full-rate PE; alternate engines by round
                gb = vpool.tile([P, nch, OUT_DIM], _bf16, tag="gb")
                if ri % 2 == 0:
                    nc.vector.tensor_copy(out=gb[:], in_=gt[:])
                else:
                    nc.scalar.activation(
                        out=gb[:], in_=gt[:],
                        func=mybir.ActivationFunctionType.Copy)

                for g in rnd["groups"]:
                    ldt0, ngch = rnd["gldt"][g]
                    gtcols = rnd["ggt"][g]
                    x = epool.tile([P, OUT_DIM], _f32, tag="x")
                    if ngch:
                        # one-hot, chunk-last: oh[k, m, c] = (ldst[k,col_c]==m)
                        # all APs keep a packed last dim -> DVE 2x mode
                        oh = opool.tile([P, P, max_gch], _bf16, tag="oh")
                        nc.vector.tensor_tensor(
                            out=oh[:, :, 0:ngch],
                            in0=_expand_mid(ldt[:, ldt0:ldt0 + ngch], P),
                            in1=it[:, :, 0:ngch],
                            op=mybir.AluOpType.is_equal)
                        acc = psum.tile([P, OUT_DIM], _f32, tag="acc")
                        for k, cg in enumerate(gtcols):
                            nc.tensor.matmul(
                                acc[:], oh[:, :, k], gb[:, cg, :],
                                start=(k == 0), stop=(k == ngch - 1))
                        nc.scalar.activation(
                            out=x[:], in_=acc[:],
                            func=mybir.ActivationFunctionType.Identity,
                            scale=norm[:, g:g + 1])
                    else:
                        nc.vector.memset(x[:], 0.0)
                    nc.vector.tensor_add(out=x[:], in0=x[:], in1=bt[:])
                    nmx = epool.tile([P, 1], _f32, tag="nmx")
                    nc.vector.tensor_reduce(out=nmx[:], in_=x[:],
                                            axis=mybir.AxisListType.X,
                                            op=mybir.AluOpType.max,
                                            negate=True)
                    nc.vector.tensor_scalar_add(
                        out=y_all[:, g * OUT_DIM:(g + 1) * OUT_DIM],
                        in0=x[:], scalar1=nmx[:, :1])
                    e = epool.tile([P, OUT_DIM], _f32, tag="e")
                    nc.scalar.activation(
                        out=e[:], in_=x[:],
                        func=mybir.ActivationFunctionType.Exp,
                        bias=nmx[:, :1], accum_out=s_all[:, g:g + 1])

            # single Ln over all groups, then finalize + batched stores
            # (8 groups per DMA via a staging tile; full GPAD rows written,
            #  host drops the 44 pad rows of the last group)
            ls_all = cpool.tile([P, NG], _f32, tag="lsall")
            nc.scalar.activation(out=ls_all[:], in_=s_all[:],
                                 func=mybir.ActivationFunctionType.Ln)
            SB = 8
            for g0 in range(0, NG, SB):
                nb = min(SB, NG - g0)
                fin = epool.tile([P, SB * OUT_DIM], _f32, tag="fin")
                for j in range(nb):
                    g = g0 + j
                    nc.vector.tensor_scalar_sub(
                        out=fin[:, j * OUT_DIM:(j + 1) * OUT_DIM],
                        in0=y_all[:, g * OUT_DIM:(g + 1) * OUT_DIM],
                        scalar1=ls_all[:, g:g + 1])
                # DRAM view [p, g, d] of out[g0:g0+nb] (strided, matches
                # fin's [p, (g d)] layout)
                oap = AP(out.ap().tensor, g0 * P * OUT_DIM,
                         [[OUT_DIM, P], [P * OUT_DIM, nb], [1, OUT_DIM]])
                nc.sync.dma_start(out=oap, in_=fin[:, :nb * OUT_DIM])
    nc.compile()
    return nc


# ------------------------------------------------------------- host prep
def _wrap_idx16(flat):
    """int16 index list -> [128, len/16] wrapped layout (16-partition groups,
    replicated across the 8 gpsimd cores)."""
    n = len(flat)
    s = n // 16
    arr = np.empty((P, s), dtype=np.int16)
    blk = flat.reshape(s, 16).T  # [16, s]
    for grp in range(8):
        arr[grp * 16:(grp + 1) * 16, :] = blk
    return arr


def prepare(h, W, b, edges):
    h = np.asarray(h, dtype=np.float32)
    W = np.asarray(W, dtype=np.float32)
    b = np.asarray(b, dtype=np.float32)
    src = np.asarray(edges[0], dtype=np.int64)
    dst = np.asarray(edges[1], dtype=np.int64)

    out_deg = np.bincount(src, minlength=N_NODES).astype(np.float32)
    in_deg = np.bincount(dst, minlength=N_NODES).astype(np.float32)

    # global m-table row for each src node (padded per-core layout)
    score = src // G
    mrow = score * GPAD + (src - score * G)
    qtab = mrow // TROWS
    lrow = (mrow - qtab * TROWS).astype(np.int16)

    dcore = dst // G
    dloc = dst - dcore * G
    grp = dloc // P
    ldst_v = (dloc - grp * P).astype(np.float32)

    # bucket = (dst-core, group, sub-table)
    bucket = (dcore * NG + grp) * NT + qtab
    order = np.argsort(bucket, kind="stable")
    bucket_s = bucket[order]
    lrow_s = lrow[order]
    ldst_s = ldst_v[order]

    nbuck = NCORES * NG * NT
    counts = np.bincount(bucket_s, minlength=nbuck).reshape(NCORES, NG, NT)
    starts = np.zeros(nbuck + 1, dtype=np.int64)
    np.cumsum(counts.reshape(-1), out=starts[1:])

    # uniform capacity per (group, sub-table): max over cores, ceil to 128
    cap = counts.max(axis=0)                      # [NG, NT]
    cap128 = ((cap + P - 1) // P) * P             # [NG, NT]

    # round structure (uniform across cores)
    # gather tile columns: (q, g, chunk) order; ldst columns: (g, q, chunk)
    rounds = []
    idx_off = 0
    ch_off = 0
    for r0 in range(0, NG, ROUND_G):
        gs = list(range(r0, min(r0 + ROUND_G, NG)))
        q_numidx, q_choff = [], []
        gt_col = {}          # (g, q) -> gather-tile column base (within round)
        cursor = 0
        for q in range(NT):
            q_choff.append(cursor)
            tot = 0
            for g in gs:
                c = int(cap128[g, q])
                gt_col[(g, q)] = cursor
                cursor += c // P
                tot += c
            q_numidx.append(tot)
        gldt, ggt = {}, {}
        lcur = 0
        for g in gs:
            cols = []
            for q in range(NT):
                cols.extend(range(gt_col[(g, q)],
                                  gt_col[(g, q)] + int(cap128[g, q]) // P))
            gldt[g] = (lcur, len(cols))
            ggt[g] = cols
            lcur += len(cols)
        rounds.append(dict(groups=gs, q_numidx=q_numidx, q_choff=q_choff,
                           nch=cursor, idx_off=idx_off, ch_off=ch_off,
                           gldt=gldt, ggt=ggt, gt_col=gt_col))
        idx_off += sum(q_numidx) // 16
        ch_off += cursor
    max_gch = max(rnd["gldt"][g][1] for rnd in rounds for g in rnd["groups"])
    meta = dict(rounds=rounds, tot_idx_cols=idx_off, tot_chunks=ch_off,
                max_gch=max_gch)

    # per-core gidx / ldst arrays
    gidx_cores = []
    ldst_cores = []
    for c in range(NCORES):
        flat_idx = np.zeros(idx_off * 16, dtype=np.int16)
        ld = np.full((P, ch_off), PAD_LDST, dtype=np.float32)
        for rnd in rounds:
            pos = rnd["idx_off"] * 16
            for q in range(NT):
                for g in rnd["groups"]:
                    bid = (c * NG + g) * NT + q
                    s0, s1 = starts[bid], starts[bid + 1]
                    n = s1 - s0
                    capq = int(cap128[g, q])
                    flat_idx[pos:pos + n] = lrow_s[s0:s1]
                    pos += capq
                    # ldst column base: group-major layout
                    qch0 = sum(int(cap128[g, q2]) // P for q2 in range(q))
                    base = rnd["ch_off"] + rnd["gldt"][g][0] + qch0
                    j = np.arange(n)
                    ld[j % P, base + j // P] = ldst_s[s0:s1]
        gidx_cores.append(_wrap_idx16(flat_idx))
        ldst_cores.append(ld.astype(ml_dtypes.bfloat16))

    # degree tiles [128, NG] (partition = node % 128 within group)
    def deg_tile(deg):
        tiles = []
        for c in range(NCORES):
            d = np.ones(GPAD, dtype=np.float32)
            d[:G] = deg[c * G:(c + 1) * G]
            tiles.append(d.reshape(NG, P).T.copy())
        return tiles

    odeg_tiles = deg_tile(out_deg)
    ideg_tiles = deg_tile(in_deg)

    hT_cores = []
    for c in range(NCORES):
        hp = np.zeros((GPAD, IN_DIM), dtype=np.float32)
        hp[:G] = h[c * G:(c + 1) * G]
        # [2, 128, GPAD]: k-halves, contiguous along nodes for wide DMAs
        ht = np.ascontiguousarray(hp.T.reshape(2, P, GPAD))
        hT_cores.append(ht)

    brep = np.broadcast_to(b, (P, OUT_DIM)).copy()
    # iotar[p, m, c] = m
    iotar = np.broadcast_to(
        np.arange(P, dtype=np.float32)[None, :, None],
        (P, P, max_gch)).astype(ml_dtypes.bfloat16).copy()

    return dict(meta=meta, gidx=gidx_cores, ldst=ldst_cores,
                odeg=odeg_tiles, ideg=ideg_tiles, hT=hT_cores,
                W=W, brep=brep, iotar=iotar)


_cache = {}


def _get_programs(meta):
    if "a" not in _cache:
        _cache["a"] = build_launch_a()
    if "b" not in _cache:
        _cache["b"] = build_launch_b(meta)
    return _cache["a"], _cache["b"]


def run_launch_a(nc_a, prep):
    in_maps = [{"hT": prep["hT"][c], "W": prep["W"], "odeg": prep["odeg"][c]}
               for c in range(NCORES)]
    res = run_bass_kernel_spmd(nc_a, in_maps, list(range(NCORES)))
    return [r["m"] for r in res.results]


def run_launch_b(nc_b, prep, m_shards):
    m_full = np.concatenate(m_shards, axis=0)  # [NCORES*GPAD, 64]
    tabs = {f"t{q}": np.ascontiguousarray(m_full[q * TROWS:(q + 1) * TROWS])
            for q in range(NT)}
    in_maps = [dict(tabs, gidx=prep["gidx"][c], ldst=prep["ldst"][c],
                    ideg=prep["ideg"][c], brep=prep["brep"],
                    iotar=prep["iotar"]) for c in range(NCORES)]
    res = run_bass_kernel_spmd(nc_b, in_maps, list(range(NCORES)))
    return np.concatenate(
        [r["out"].reshape(GPAD, OUT_DIM)[:G] for r in res.results], axis=0)


def kernel(h, W, b, edges):
    prep = prepare(h, W, b, edges)
    nc_a, nc_b = _get_programs(prep["meta"])
    m_shards = run_launch_a(nc_a, prep)
    out = run_launch_b(nc_b, prep, m_shards)
    return out.astype(np.float32)
